# revision 28
# baseline (speedup 1.0000x reference)
"""Trainium2 Bass kernel: autoregressive flow layer (masked-GEMM formulation).

Math (reference):
    zp = z[:, perm]
    s  = tanh(zp @ (W_s*tril(-1)).T + b_s);  s[:,0] = s0
    m  = tanh(zp @ (W_m*tril(-1)).T + b_m);  m[:,0] = m0
    x  = zp * exp(s) + m;   logdet = sum(s, axis=1)

Device formulation (perm folded into constant weights on host):
    A~ = (W*mask)[pinv,:][:,pinv].T, b~ = b[pinv], i* = perm[0]
    s~ = tanh(z @ A~_s + b~_s);  s~[:, i*] = s0   (same for m~)
    x~ = z * exp(s~) + m~
    x  = x~ @ P  with P[perm[j], j] = 1   (permute+store)
    logdet = sum_i s~[:, i]               (ones-matmul, perm-invariant)

Per-core layout: transposed compute ([feature(p=128 tiles), batch(f)]),
batch sharded over 8 cores, weights replicated. On-device per core:
z cast (DVE) + 64 PE block-transposes, 2 masked GEMMs (bf16, f32 accum),
tanh/exp with fused per-partition bias (ACT), elementwise (DVE), ones-GEMM
for logdet, and a one-hot permute-GEMM producing x in natural layout.

Two environment workarounds (see _split_waits): neuronxcc's walrus rejects
instructions with >1 attached sync wait, so Tile's multi-waits are hoisted
onto standalone EventSemaphore instructions before compile.
"""

import numpy as np
import ml_dtypes

B, D = 8192, 1024
N_CORES = 8
B_LOC = B // N_CORES  # 1024 rows per core
NT = D // 128  # 8 feature tiles

LAST_RESULT = None  # BassKernelResults of the most recent run (for profiling)

# "gemm": permute+transpose x via a one-hot matmul on the TensorEngine
# "gather": store x~T to DRAM, indirect-DMA row-gather by perm, PE transpose back
OUTPUT_MODE = "gather"


def _build_nc(output_mode: str | None = None):
    if output_mode is None:
        output_mode = OUTPUT_MODE
    import concourse.bass as bass
    import concourse.mybir as mybir
    import concourse.tile as tile
    from concourse.masks import make_identity

    f32 = mybir.dt.float32
    bf16 = mybir.dt.bfloat16
    AF = mybir.ActivationFunctionType

    nc = bass.Bass()
    z_in = nc.declare_dram_parameter("z", [B_LOC, D], f32, isOutput=False)
    a_s_in = nc.declare_dram_parameter("a_s", [D, D], bf16, isOutput=False)
    a_m_in = nc.declare_dram_parameter("a_m", [D, D], bf16, isOutput=False)
    if output_mode == "gemm":
        p_in = nc.declare_dram_parameter("p", [D, D], bf16, isOutput=False)
    else:
        gidx_in = nc.declare_dram_parameter("gidx", [128, NT], mybir.dt.int32,
                                            isOutput=False)
    bt_s_in = nc.declare_dram_parameter("bt_s", [128, NT], f32, isOutput=False)
    bt_m_in = nc.declare_dram_parameter("bt_m", [128, NT], f32, isOutput=False)
    x_out = nc.declare_dram_parameter("x", [B_LOC, D], f32, isOutput=True)
    ld_out = nc.declare_dram_parameter("ld", [1, B_LOC], f32, isOutput=True)

    with tile.TileContext(nc) as tc:
        with (
            tc.tile_pool(name="persist", bufs=1) as pp,
            tc.tile_pool(name="stage", bufs=3) as st,
            tc.tile_pool(name="psum", bufs=1, space="PSUM") as psp,
        ):
            # --- constants ---
            ident = pp.tile([128, 128], f32, tag="ident", name="ident")
            make_identity(nc, ident)
            ident_bf = pp.tile([128, 128], bf16, tag="identbf", name="ident_bf")
            make_identity(nc, ident_bf)
            ones = pp.tile([128, 1], bf16, tag="ones", name="ones")
            nc.gpsimd.memset(ones, 1.0)
            bias_s = pp.tile([128, NT], f32, tag="bias_s", name="bias_s")
            nc.sync.dma_start(out=bias_s, in_=bt_s_in[:, :])
            bias_m = pp.tile([128, NT], f32, tag="bias_m", name="bias_m")
            nc.sync.dma_start(out=bias_m, in_=bt_m_in[:, :])


            # --- z load + cast to bf16 (bf16 transposes are 2x cheaper on PE) ---
            zb = []
            for t in range(NT):
                zf = st.tile([128, D], f32, tag="zf", bufs=3, name=f"zf{t}")
                nc.sync.dma_start(out=zf, in_=z_in[t * 128:(t + 1) * 128, :])
                zbt = pp.tile([128, D], bf16, tag=f"zb{t}", name=f"zb{t}")
                nc.vector.tensor_copy(zbt, zf)
                zb.append(zbt)

            # --- weights (replicated) ---
            As, Am, Pt = [], [], []
            for t in range(NT):
                a = pp.tile([128, D], bf16, tag=f"as{t}", name=f"as{t}")
                nc.sync.dma_start(out=a, in_=a_s_in[t * 128:(t + 1) * 128, :])
                As.append(a)
            for t in range(NT):
                a = pp.tile([128, D], bf16, tag=f"am{t}", name=f"am{t}")
                nc.sync.dma_start(out=a, in_=a_m_in[t * 128:(t + 1) * 128, :])
                Am.append(a)
            if output_mode == "gemm":
                for t in range(NT):
                    a = pp.tile([128, D], bf16, tag=f"pp{t}", name=f"pt{t}")
                    nc.sync.dma_start(out=a, in_=p_in[t * 128:(t + 1) * 128, :])
                    Pt.append(a)
            else:
                gidx_sb = pp.tile([128, NT], mybir.dt.int32, tag="gidx",
                                  name="gidx_sb")
                nc.sync.dma_start(out=gidx_sb, in_=gidx_in[:, :])

            # --- transpose z -> zT tiles [128(i), B_LOC(b)] bf16 ---
            zT = [
                pp.tile([128, B_LOC], bf16, tag=f"zt{t}", name=f"zt{t}")
                for t in range(NT)
            ]
            for tg in range(2):
                for ti in range(NT):
                    pt = psp.tile([128, 512], bf16, tag="g", bufs=4,
                                  name=f"ptr{ti}_{tg}")
                    for q in range(4):
                        tb = tg * 4 + q
                        nc.tensor.transpose(
                            pt[:, q * 128:(q + 1) * 128],
                            zb[tb][:, ti * 128:(ti + 1) * 128],
                            ident_bf,
                        )
                    dst = zT[ti][:, tg * 512:(tg + 1) * 512]
                    if ti % 2 == 0:
                        nc.vector.tensor_copy(dst, pt)
                    else:
                        nc.scalar.copy(dst, pt)

            # --- main GEMMs + activations, transposed layout ---
            s_t = [
                pp.tile([128, B_LOC], bf16, tag=f"s{t}", name=f"s{t}")
                for t in range(NT)
            ]
            m_t = [
                pp.tile([128, B_LOC], bf16, tag=f"m{t}", name=f"m{t}")
                for t in range(NT)
            ]
            ld_ps = [
                psp.tile([1, 512], f32, tag=f"ld{c}", bufs=1, name=f"ld{c}")
                for c in range(2)
            ]
            x_t = []
            for it in range(NT):
                for c in range(2):
                    cs = slice(c * 512, (c + 1) * 512)
                    g = psp.tile([128, 512], f32, tag="g", bufs=4,
                                 name=f"gs{it}_{c}")
                    for k in range(NT):
                        nc.tensor.matmul(
                            g, As[k][:, it * 128:(it + 1) * 128], zT[k][:, cs],
                            start=(k == 0), stop=(k == NT - 1),
                        )
                    nc.scalar.activation(
                        s_t[it][:, cs], g, AF.Tanh, bias=bias_s[:, it:it + 1]
                    )
                    g2 = psp.tile([128, 512], f32, tag="g", bufs=4,
                                  name=f"gm{it}_{c}")
                    for k in range(NT):
                        nc.tensor.matmul(
                            g2, Am[k][:, it * 128:(it + 1) * 128], zT[k][:, cs],
                            start=(k == 0), stop=(k == NT - 1),
                        )
                    nc.scalar.activation(
                        m_t[it][:, cs], g2, AF.Tanh, bias=bias_m[:, it:it + 1]
                    )
                for c in range(2):
                    cs = slice(c * 512, (c + 1) * 512)
                    nc.tensor.matmul(
                        ld_ps[c], ones, s_t[it][:, cs],
                        start=(it == 0), stop=(it == NT - 1),
                    )
                e = pp.tile([128, B_LOC], bf16, tag=f"e{it}", name=f"e{it}")
                nc.scalar.activation(e, s_t[it], AF.Exp)
                xt = pp.tile([128, B_LOC], bf16, tag=f"x{it}", name=f"x{it}")
                nc.vector.tensor_mul(xt, e, zT[it])
                nc.vector.tensor_add(xt, xt, m_t[it])
                x_t.append(xt)

            if output_mode == "gemm":
                # --- output permute GEMM: x[b, j] = sum_i x~T[i, b] P[i, j] ---
                for tb in range(NT):
                    for c in range(2):
                        cs = slice(c * 512, (c + 1) * 512)
                        gx = psp.tile([128, 512], f32, tag="gx", bufs=2,
                                      name=f"gx{tb}_{c}")
                        for it in range(NT):
                            nc.tensor.matmul(
                                gx, x_t[it][:, tb * 128:(tb + 1) * 128],
                                Pt[it][:, cs],
                                start=(it == 0), stop=(it == NT - 1),
                            )
                        xo = st.tile([128, 512], f32, tag="xo",
                                     name=f"xo{tb}_{c}")
                        nc.vector.tensor_copy(xo, gx)
                        nc.sync.dma_start(
                            out=x_out[tb * 128:(tb + 1) * 128, cs], in_=xo
                        )
            else:
                # --- output via DRAM round-trip: store x~T, row-gather by
                # perm (x^T), PE-transpose back to natural layout ---
                with tc.tile_pool(name="dram", bufs=1, space="DRAM") as dp:
                    xT_dram = dp.tile([D, B_LOC], bf16, tag="xtd",
                                      name="xt_dram")
                    for it in range(NT):
                        nc.sync.dma_start(
                            out=xT_dram[it * 128:(it + 1) * 128, :],
                            in_=x_t[it],
                        )
                    xg = []
                    for jt in range(NT):
                        g = pp.tile([128, B_LOC], bf16, tag=f"xg{jt}",
                                    name=f"xg{jt}")
                        nc.gpsimd.indirect_dma_start(
                            out=g[:, :],
                            out_offset=None,
                            in_=xT_dram[:, :],
                            in_offset=bass.IndirectOffsetOnAxis(
                                ap=gidx_sb[:, jt:jt + 1], axis=0
                            ),
                        )
                        xg.append(g)
                    for tb in range(NT):
                        for g2 in range(2):
                            pt = psp.tile([128, 512], bf16, tag="g", bufs=4,
                                          name=f"ptx{tb}_{g2}")
                            for q in range(4):
                                jt = g2 * 4 + q
                                nc.tensor.transpose(
                                    pt[:, q * 128:(q + 1) * 128],
                                    xg[jt][:, tb * 128:(tb + 1) * 128],
                                    ident_bf,
                                )
                            xo = st.tile([128, 512], f32, tag="xo",
                                         name=f"xo{tb}_{g2}")
                            nc.vector.tensor_copy(xo, pt)
                            nc.sync.dma_start(
                                out=x_out[tb * 128:(tb + 1) * 128,
                                          g2 * 512:(g2 + 1) * 512],
                                in_=xo,
                            )

            # --- logdet out ---
            ldsb = st.tile([1, B_LOC], f32, tag="ldsb", name="ldsb")
            for c in range(2):
                nc.vector.tensor_copy(ldsb[0:1, c * 512:(c + 1) * 512], ld_ps[c])
            nc.sync.dma_start(out=ld_out[0:1, :], in_=ldsb)

    return nc


def _split_waits(nc, max_waits=1):
    """neuronxcc's walrus rejects instructions with more than ~1 attached
    sync wait ("Too many sync wait commands"). Hoist excess waits onto
    standalone EventSemaphore instructions on the same engine, immediately
    before the instruction (engine FIFO makes this equivalent)."""
    import concourse.mybir as mybir

    n = 0
    for fn in nc.m.functions:
        for blk in fn.blocks:
            new = []
            for inst in blk.instructions:
                si = getattr(inst, "sync_info", None)
                waits = list(si.on_wait) if (si and si.on_wait) else []
                if len(waits) > max_waits and inst.opcode != "EventSemaphore":
                    for w in waits:
                        n += 1
                        new.append(mybir.InstEventSemaphore(
                            name=f"{inst.name}_hw{n}",
                            engine=inst.engine,
                            ins=[],
                            outs=[],
                            sync_info=mybir.SyncInfo(on_wait=[w], on_update=[]),
                        ))
                    si.on_wait = []
                new.append(inst)
            blk.instructions[:] = new
    return nc


def _prepare(z, perm, W_s, b_s, W_m, b_m, s0, m0):
    """Host-side constant folding + sharding.

    Returns (in_maps, fixup) where fixup is None, or (i_star, s0v, m0v)
    when the j=0 column must be patched on the host (|s0| or |m0| >= 1,
    outside tanh's range so it can't be folded into the bias).
    """
    z = np.asarray(z, dtype=np.float32)
    perm = np.asarray(perm).astype(np.int64)
    W_s = np.asarray(W_s, dtype=np.float32)
    W_m = np.asarray(W_m, dtype=np.float32)
    b_s = np.asarray(b_s, dtype=np.float32).copy()
    b_m = np.asarray(b_m, dtype=np.float32).copy()
    s0v = float(np.asarray(s0).reshape(-1)[0])
    m0v = float(np.asarray(m0).reshape(-1)[0])

    pinv = np.argsort(perm)
    mask = np.tril(np.ones((D, D), np.float32), k=-1)
    bf = ml_dtypes.bfloat16
    A_s = np.ascontiguousarray((W_s * mask)[np.ix_(pinv, pinv)].T).astype(bf)
    A_m = np.ascontiguousarray((W_m * mask)[np.ix_(pinv, pinv)].T).astype(bf)
    P = np.zeros((D, D), np.float32)
    P[perm, np.arange(D)] = 1.0
    P = P.astype(bf)
    gidx = np.ascontiguousarray(
        perm.reshape(NT, 128).T.astype(np.int32))  # gidx[p, jt] = perm[jt*128+p]
    i_star = int(perm[0])

    # column j=0 uses the raw s0/m0 rather than the conditioner. The i*
    # column of A~ is already all-zero (strict-lower row 0), so
    # s~[:, i*] = tanh(b~[i*]); fold s0 = tanh(b~[i*]) when possible.
    bts = b_s[pinv]
    btm = b_m[pinv]
    fixup = None
    if abs(s0v) < 0.999 and abs(m0v) < 0.999:
        bts[i_star] = np.arctanh(s0v)
        btm[i_star] = np.arctanh(m0v)
    else:
        bts[i_star] = 0.0
        btm[i_star] = 0.0
        fixup = (i_star, s0v, m0v)
    bt_s = np.ascontiguousarray(bts.reshape(NT, 128).T)
    bt_m = np.ascontiguousarray(btm.reshape(NT, 128).T)

    in_maps = []
    for c in range(N_CORES):
        in_maps.append({
            "z": np.ascontiguousarray(z[c * B_LOC:(c + 1) * B_LOC]),
            "a_s": A_s, "a_m": A_m, "p": P, "gidx": gidx,
            "bt_s": bt_s, "bt_m": bt_m,
        })
    return in_maps, fixup, z


def kernel(z, perm, W_s, b_s, W_m, b_m, s0, m0):
    global LAST_RESULT
    from concourse.bass_utils import run_bass_kernel_spmd

    in_maps, fixup, z_np = _prepare(z, perm, W_s, b_s, W_m, b_m, s0, m0)
    nc = _split_waits(_build_nc())
    res = run_bass_kernel_spmd(nc, in_maps, core_ids=list(range(N_CORES)))
    LAST_RESULT = res
    x = np.concatenate([np.asarray(r["x"], dtype=np.float32)
                        for r in res.results], axis=0)
    ld = np.concatenate([np.asarray(r["ld"], dtype=np.float32).reshape(-1)
                         for r in res.results], axis=0)
    if fixup is not None:
        i_star, s0v, m0v = fixup
        # device computed s~[:,i*] = m~[:,i*] = 0: x[:,0] = zp0, ld unchanged
        x[:, 0] = z_np[:, i_star] * np.exp(np.float32(s0v)) + np.float32(m0v)
        ld = ld + np.float32(s0v)
    return x, ld


# revision 31
# speedup vs baseline: 1.0397x; 1.0397x over previous
"""Trainium2 Bass kernel: autoregressive flow layer (masked-GEMM formulation).

Math (reference):
    zp = z[:, perm]
    s  = tanh(zp @ (W_s*tril(-1)).T + b_s);  s[:,0] = s0
    m  = tanh(zp @ (W_m*tril(-1)).T + b_m);  m[:,0] = m0
    x  = zp * exp(s) + m;   logdet = sum(s, axis=1)

Device formulation (perm folded into constant weights on host):
    A~ = (W*mask)[pinv,:][:,pinv].T, b~ = b[pinv], i* = perm[0]
    s~ = tanh(z @ A~_s + b~_s);  s~[:, i*] = s0   (same for m~)
    x~ = z * exp(s~) + m~
    x  = x~ @ P  with P[perm[j], j] = 1   (permute+store)
    logdet = sum_i s~[:, i]               (ones-matmul, perm-invariant)

Per-core layout: transposed compute ([feature(p=128 tiles), batch(f)]),
batch sharded over 8 cores, weights replicated. On-device per core:
z cast (DVE) + 64 PE block-transposes, 2 masked GEMMs (bf16, f32 accum),
tanh/exp with fused per-partition bias (ACT), elementwise (DVE), ones-GEMM
for logdet, and a one-hot permute-GEMM producing x in natural layout.

Two environment workarounds (see _split_waits): neuronxcc's walrus rejects
instructions with >1 attached sync wait, so Tile's multi-waits are hoisted
onto standalone EventSemaphore instructions before compile.
"""

import numpy as np
import ml_dtypes

B, D = 8192, 1024
N_CORES = 8
B_LOC = B // N_CORES  # 1024 rows per core
NT = D // 128  # 8 feature tiles

LAST_RESULT = None  # BassKernelResults of the most recent run (for profiling)

# "gemm": permute+transpose x via a one-hot matmul on the TensorEngine
# "gather": store x~T to DRAM, indirect-DMA row-gather by perm, PE transpose back
OUTPUT_MODE = "gather"


def _build_nc(output_mode: str | None = None):
    if output_mode is None:
        output_mode = OUTPUT_MODE
    import concourse.bass as bass
    import concourse.mybir as mybir
    import concourse.tile as tile
    from concourse.masks import make_identity

    f32 = mybir.dt.float32
    bf16 = mybir.dt.bfloat16
    AF = mybir.ActivationFunctionType

    nc = bass.Bass()
    z_in = nc.declare_dram_parameter("z", [B_LOC, D], f32, isOutput=False)
    a_s_in = nc.declare_dram_parameter("a_s", [D, D], bf16, isOutput=False)
    a_m_in = nc.declare_dram_parameter("a_m", [D, D], bf16, isOutput=False)
    if output_mode == "gemm":
        p_in = nc.declare_dram_parameter("p", [D, D], bf16, isOutput=False)
    else:
        gidx_in = nc.declare_dram_parameter("gidx", [128, NT], mybir.dt.int32,
                                            isOutput=False)
    bt_s_in = nc.declare_dram_parameter("bt_s", [128, NT], f32, isOutput=False)
    bt_m_in = nc.declare_dram_parameter("bt_m", [128, NT], f32, isOutput=False)
    x_out = nc.declare_dram_parameter("x", [B_LOC, D], f32, isOutput=True)
    ld_out = nc.declare_dram_parameter("ld", [1, B_LOC], f32, isOutput=True)

    with tile.TileContext(nc) as tc:
        with (
            tc.tile_pool(name="persist", bufs=1) as pp,
            tc.tile_pool(name="stage", bufs=3) as st,
            tc.tile_pool(name="psum", bufs=1, space="PSUM") as psp,
        ):
            # --- constants ---
            ident = pp.tile([128, 128], f32, tag="ident", name="ident")
            make_identity(nc, ident)
            ident_bf = pp.tile([128, 128], bf16, tag="identbf", name="ident_bf")
            make_identity(nc, ident_bf)
            ones = pp.tile([128, 1], bf16, tag="ones", name="ones")
            nc.gpsimd.memset(ones, 1.0)
            bias_s = pp.tile([128, NT], f32, tag="bias_s", name="bias_s")
            nc.sync.dma_start(out=bias_s, in_=bt_s_in[:, :])
            bias_m = pp.tile([128, NT], f32, tag="bias_m", name="bias_m")
            nc.sync.dma_start(out=bias_m, in_=bt_m_in[:, :])


            # --- z load (f32; transposed on PE directly, cast on evacuation) ---
            zb = []
            for t in range(NT):
                zf = st.tile([128, D], f32, tag=f"zf{t}", bufs=1, name=f"zf{t}")
                nc.sync.dma_start(out=zf, in_=z_in[t * 128:(t + 1) * 128, :])
                zb.append(zf)

            # --- weights (replicated) ---
            As, Am, Pt = [], [], []
            for t in range(NT):
                a = pp.tile([128, D], bf16, tag=f"as{t}", name=f"as{t}")
                nc.sync.dma_start(out=a, in_=a_s_in[t * 128:(t + 1) * 128, :])
                As.append(a)
            for t in range(NT):
                a = pp.tile([128, D], bf16, tag=f"am{t}", name=f"am{t}")
                nc.sync.dma_start(out=a, in_=a_m_in[t * 128:(t + 1) * 128, :])
                Am.append(a)
            if output_mode == "gemm":
                for t in range(NT):
                    a = pp.tile([128, D], bf16, tag=f"pp{t}", name=f"pt{t}")
                    nc.sync.dma_start(out=a, in_=p_in[t * 128:(t + 1) * 128, :])
                    Pt.append(a)
            else:
                gidx_sb = pp.tile([128, NT], mybir.dt.int32, tag="gidx",
                                  name="gidx_sb")
                nc.sync.dma_start(out=gidx_sb, in_=gidx_in[:, :])

            # --- transpose z -> zT tiles [128(i), B_LOC(b)] bf16 ---
            zT = [
                pp.tile([128, B_LOC], bf16, tag=f"zt{t}", name=f"zt{t}")
                for t in range(NT)
            ]
            for tg in range(2):
                for ti in range(NT):
                    pt = psp.tile([128, 512], f32, tag="g", bufs=4,
                                  name=f"ptr{ti}_{tg}")
                    for q in range(4):
                        tb = tg * 4 + q
                        nc.tensor.transpose(
                            pt[:, q * 128:(q + 1) * 128],
                            zb[tb][:, ti * 128:(ti + 1) * 128],
                            ident,
                        )
                    dst = zT[ti][:, tg * 512:(tg + 1) * 512]
                    if ti % 2 == 0:
                        nc.vector.tensor_copy(dst, pt)
                    else:
                        nc.scalar.copy(dst, pt)

            # --- main GEMMs + activations, transposed layout ---
            s_t = [
                pp.tile([128, B_LOC], bf16, tag=f"s{t}", name=f"s{t}")
                for t in range(NT)
            ]
            m_t = [
                pp.tile([128, B_LOC], bf16, tag=f"m{t}", name=f"m{t}")
                for t in range(NT)
            ]
            ld_ps = [
                psp.tile([1, 512], f32, tag=f"ld{c}", bufs=1, name=f"ld{c}")
                for c in range(2)
            ]
            x_t = []
            for it in range(NT):
                for c in range(2):
                    cs = slice(c * 512, (c + 1) * 512)
                    g = psp.tile([128, 512], f32, tag="g", bufs=4,
                                 name=f"gs{it}_{c}")
                    for k in range(NT):
                        nc.tensor.matmul(
                            g, As[k][:, it * 128:(it + 1) * 128], zT[k][:, cs],
                            start=(k == 0), stop=(k == NT - 1),
                        )
                    nc.scalar.activation(
                        s_t[it][:, cs], g, AF.Tanh, bias=bias_s[:, it:it + 1]
                    )
                    g2 = psp.tile([128, 512], f32, tag="g", bufs=4,
                                  name=f"gm{it}_{c}")
                    for k in range(NT):
                        nc.tensor.matmul(
                            g2, Am[k][:, it * 128:(it + 1) * 128], zT[k][:, cs],
                            start=(k == 0), stop=(k == NT - 1),
                        )
                    nc.scalar.activation(
                        m_t[it][:, cs], g2, AF.Tanh, bias=bias_m[:, it:it + 1]
                    )
                for c in range(2):
                    cs = slice(c * 512, (c + 1) * 512)
                    nc.tensor.matmul(
                        ld_ps[c], ones, s_t[it][:, cs],
                        start=(it == 0), stop=(it == NT - 1),
                    )
                e = pp.tile([128, B_LOC], bf16, tag=f"e{it}", name=f"e{it}")
                nc.scalar.activation(e, s_t[it], AF.Exp)
                xt = pp.tile([128, B_LOC], bf16, tag=f"x{it}", name=f"x{it}")
                nc.vector.tensor_mul(xt, e, zT[it])
                nc.vector.tensor_add(xt, xt, m_t[it])
                x_t.append(xt)

            if output_mode == "gemm":
                # --- output permute GEMM: x[b, j] = sum_i x~T[i, b] P[i, j] ---
                for tb in range(NT):
                    for c in range(2):
                        cs = slice(c * 512, (c + 1) * 512)
                        gx = psp.tile([128, 512], f32, tag="g", bufs=4,
                                      name=f"gx{tb}_{c}")
                        for it in range(NT):
                            nc.tensor.matmul(
                                gx, x_t[it][:, tb * 128:(tb + 1) * 128],
                                Pt[it][:, cs],
                                start=(it == 0), stop=(it == NT - 1),
                            )
                        xo = st.tile([128, 512], f32, tag="xo",
                                     name=f"xo{tb}_{c}")
                        nc.vector.tensor_copy(xo, gx)
                        nc.sync.dma_start(
                            out=x_out[tb * 128:(tb + 1) * 128, cs], in_=xo
                        )
            else:
                # --- output via DRAM round-trip: store x~T, row-gather by
                # perm (x^T), PE-transpose back to natural layout ---
                with tc.tile_pool(name="dram", bufs=1, space="DRAM") as dp:
                    xT_dram = dp.tile([D, B_LOC], bf16, tag="xtd",
                                      name="xt_dram")
                    for it in range(NT):
                        nc.sync.dma_start(
                            out=xT_dram[it * 128:(it + 1) * 128, :],
                            in_=x_t[it],
                        )
                    xg = []
                    for jt in range(NT):
                        g = pp.tile([128, B_LOC], bf16, tag=f"xg{jt}",
                                    name=f"xg{jt}")
                        nc.gpsimd.indirect_dma_start(
                            out=g[:, :],
                            out_offset=None,
                            in_=xT_dram[:, :],
                            in_offset=bass.IndirectOffsetOnAxis(
                                ap=gidx_sb[:, jt:jt + 1], axis=0
                            ),
                        )
                        xg.append(g)
                    for tb in range(NT):
                        for g2 in range(2):
                            pt = psp.tile([128, 512], bf16, tag="g", bufs=4,
                                          name=f"ptx{tb}_{g2}")
                            for q in range(4):
                                jt = g2 * 4 + q
                                nc.tensor.transpose(
                                    pt[:, q * 128:(q + 1) * 128],
                                    xg[jt][:, tb * 128:(tb + 1) * 128],
                                    ident_bf,
                                )
                            xo = st.tile([128, 512], f32, tag="xo",
                                         name=f"xo{tb}_{g2}")
                            nc.vector.tensor_copy(xo, pt)
                            nc.sync.dma_start(
                                out=x_out[tb * 128:(tb + 1) * 128,
                                          g2 * 512:(g2 + 1) * 512],
                                in_=xo,
                            )

            # --- logdet out ---
            ldsb = st.tile([1, B_LOC], f32, tag="ldsb", name="ldsb")
            for c in range(2):
                nc.vector.tensor_copy(ldsb[0:1, c * 512:(c + 1) * 512], ld_ps[c])
            nc.sync.dma_start(out=ld_out[0:1, :], in_=ldsb)

    return nc


def _split_waits(nc, max_waits=1):
    """neuronxcc's walrus rejects instructions with more than ~1 attached
    sync wait ("Too many sync wait commands"). Hoist excess waits onto
    standalone EventSemaphore instructions on the same engine, immediately
    before the instruction (engine FIFO makes this equivalent)."""
    import concourse.mybir as mybir

    n = 0
    for fn in nc.m.functions:
        for blk in fn.blocks:
            new = []
            for inst in blk.instructions:
                si = getattr(inst, "sync_info", None)
                waits = list(si.on_wait) if (si and si.on_wait) else []
                if len(waits) > max_waits and inst.opcode != "EventSemaphore":
                    for w in waits:
                        n += 1
                        new.append(mybir.InstEventSemaphore(
                            name=f"{inst.name}_hw{n}",
                            engine=inst.engine,
                            ins=[],
                            outs=[],
                            sync_info=mybir.SyncInfo(on_wait=[w], on_update=[]),
                        ))
                    si.on_wait = []
                new.append(inst)
            blk.instructions[:] = new
    return nc


def _prepare(z, perm, W_s, b_s, W_m, b_m, s0, m0):
    """Host-side constant folding + sharding.

    Returns (in_maps, fixup) where fixup is None, or (i_star, s0v, m0v)
    when the j=0 column must be patched on the host (|s0| or |m0| >= 1,
    outside tanh's range so it can't be folded into the bias).
    """
    z = np.asarray(z, dtype=np.float32)
    perm = np.asarray(perm).astype(np.int64)
    W_s = np.asarray(W_s, dtype=np.float32)
    W_m = np.asarray(W_m, dtype=np.float32)
    b_s = np.asarray(b_s, dtype=np.float32).copy()
    b_m = np.asarray(b_m, dtype=np.float32).copy()
    s0v = float(np.asarray(s0).reshape(-1)[0])
    m0v = float(np.asarray(m0).reshape(-1)[0])

    pinv = np.argsort(perm)
    mask = np.tril(np.ones((D, D), np.float32), k=-1)
    bf = ml_dtypes.bfloat16
    A_s = np.ascontiguousarray((W_s * mask)[np.ix_(pinv, pinv)].T).astype(bf)
    A_m = np.ascontiguousarray((W_m * mask)[np.ix_(pinv, pinv)].T).astype(bf)
    P = np.zeros((D, D), np.float32)
    P[perm, np.arange(D)] = 1.0
    P = P.astype(bf)
    gidx = np.ascontiguousarray(
        perm.reshape(NT, 128).T.astype(np.int32))  # gidx[p, jt] = perm[jt*128+p]
    i_star = int(perm[0])

    # column j=0 uses the raw s0/m0 rather than the conditioner. The i*
    # column of A~ is already all-zero (strict-lower row 0), so
    # s~[:, i*] = tanh(b~[i*]); fold s0 = tanh(b~[i*]) when possible.
    bts = b_s[pinv]
    btm = b_m[pinv]
    fixup = None
    if abs(s0v) < 0.999 and abs(m0v) < 0.999:
        bts[i_star] = np.arctanh(s0v)
        btm[i_star] = np.arctanh(m0v)
    else:
        bts[i_star] = 0.0
        btm[i_star] = 0.0
        fixup = (i_star, s0v, m0v)
    bt_s = np.ascontiguousarray(bts.reshape(NT, 128).T)
    bt_m = np.ascontiguousarray(btm.reshape(NT, 128).T)

    in_maps = []
    for c in range(N_CORES):
        in_maps.append({
            "z": np.ascontiguousarray(z[c * B_LOC:(c + 1) * B_LOC]),
            "a_s": A_s, "a_m": A_m, "p": P, "gidx": gidx,
            "bt_s": bt_s, "bt_m": bt_m,
        })
    return in_maps, fixup, z


def kernel(z, perm, W_s, b_s, W_m, b_m, s0, m0):
    global LAST_RESULT
    from concourse.bass_utils import run_bass_kernel_spmd

    in_maps, fixup, z_np = _prepare(z, perm, W_s, b_s, W_m, b_m, s0, m0)
    nc = _split_waits(_build_nc())
    res = run_bass_kernel_spmd(nc, in_maps, core_ids=list(range(N_CORES)))
    LAST_RESULT = res
    x = np.concatenate([np.asarray(r["x"], dtype=np.float32)
                        for r in res.results], axis=0)
    ld = np.concatenate([np.asarray(r["ld"], dtype=np.float32).reshape(-1)
                         for r in res.results], axis=0)
    if fixup is not None:
        i_star, s0v, m0v = fixup
        # device computed s~[:,i*] = m~[:,i*] = 0: x[:,0] = zp0, ld unchanged
        x[:, 0] = z_np[:, i_star] * np.exp(np.float32(s0v)) + np.float32(m0v)
        ld = ld + np.float32(s0v)
    return x, ld
